# revision 27
# baseline (speedup 1.0000x reference)
"""RWKV WKV attention kernel for TRN2 (Bass/Tile), batch-parallel over 8 cores.

v3: single fused pass, bf16 matmuls/elementwise, host-precomputed transposed
time-mix inputs (no on-device transposes or mixes), inclusive scans via the
identity num' = a*(ew*e^u - 1) + sa_incl = ew*num (ew cancels in num/den),
ACT-assisted scaling, direct PSUM->DRAM output DMA.

Per core (one batch element, D=1024, T=4096, chunks of TC=512):
  host: xmk/xmv/xmr = time-mixed x, transposed to [D, T], bf16.
  per chunk c, per channel-group e (128 ch):
    k = Wk @ xmk, v = Wv @ xmv (PE, bf16, PSUM fp32)
    ek = exp(k) [ACT], vs = v [ACT copy bf16]
    a = ek*vs [DVE]
    sa = scan(ew, a), sb = scan(ew, ek)  (inclusive, DVE)
    ac = a*c, ekc = ek*c [ACT copy w/ scale], c = ew*e^u - 1
    num = ac + sa, den = ekc + sb [DVE]; rden = 1/den [DVE fp32]
    wkv = num * rden [Pool]
    r = Wr @ xmr (PE); sr = sigmoid(r) [ACT, batched per chunk]
    rw = wkv * sr [Pool]
  out(c) = rw(c)^T @ Wo^T (PE, next-chunk slot), DMA PSUM -> DRAM.

Host-packed weights [128, 8*1024] bf16: arr[p, j*1024+e] = W[e, j*128+p].
cv [128, 16] fp32: col j of each group = channels j*128..j*128+127:
  0-7 ew = exp(-exp(time_decay)), 8-15 c = ew*exp(time_first) - 1.
"""
import sys
for p in ("/opt/trn_rl_repo",):
    if p not in sys.path:
        sys.path.insert(0, p)

import numpy as np
from contextlib import ExitStack

import concourse.bass as bass
import concourse.tile as tile
from concourse import bacc, mybir

dt = mybir.dt
AF = mybir.ActivationFunctionType
OP = mybir.AluOpType

D = 1024
NJ = D // 128  # 8 channel groups


def build(nc, T=4096, TC=512):
    nch = T // TC
    NTS = TC // 128

    XK = nc.dram_tensor("xmk", [D, T], dt.bfloat16, kind="ExternalInput").ap()
    XV = nc.dram_tensor("xmv", [D, T], dt.bfloat16, kind="ExternalInput").ap()
    XR = nc.dram_tensor("xmr", [D, T], dt.bfloat16, kind="ExternalInput").ap()
    WK = nc.dram_tensor("wk", [128, NJ * D], dt.bfloat16, kind="ExternalInput").ap()
    WV = nc.dram_tensor("wv", [128, NJ * D], dt.bfloat16, kind="ExternalInput").ap()
    WR = nc.dram_tensor("wr", [128, NJ * D], dt.bfloat16, kind="ExternalInput").ap()
    WO = nc.dram_tensor("wo", [128, NJ * D], dt.bfloat16, kind="ExternalInput").ap()
    CV = nc.dram_tensor("cv", [128, 16], dt.float32, kind="ExternalInput").ap()
    O = nc.dram_tensor("o", [T, D], dt.bfloat16, kind="ExternalOutput").ap()

    with tile.TileContext(nc) as tc, ExitStack() as ctx:
        wp = ctx.enter_context(tc.tile_pool(name="wp", bufs=1))
        xmp = ctx.enter_context(tc.tile_pool(name="xmp", bufs=2))
        kps = ctx.enter_context(tc.tile_pool(name="kps", bufs=2, space="PSUM"))
        vps = ctx.enter_context(tc.tile_pool(name="vps", bufs=2, space="PSUM"))
        rps = ctx.enter_context(tc.tile_pool(name="rps", bufs=2, space="PSUM"))
        ops_ = ctx.enter_context(tc.tile_pool(name="ops", bufs=2, space="PSUM"))
        ekp = ctx.enter_context(tc.tile_pool(name="ekp", bufs=3))
        vsp = ctx.enter_context(tc.tile_pool(name="vsp", bufs=3))
        ap_ = ctx.enter_context(tc.tile_pool(name="ap", bufs=3))
        sap = ctx.enter_context(tc.tile_pool(name="sap", bufs=3))
        sbp = ctx.enter_context(tc.tile_pool(name="sbp", bufs=3))
        acp = ctx.enter_context(tc.tile_pool(name="acp", bufs=2))
        ekcp = ctx.enter_context(tc.tile_pool(name="ekcp", bufs=2))
        nump = ctx.enter_context(tc.tile_pool(name="nump", bufs=2))
        denp = ctx.enter_context(tc.tile_pool(name="denp", bufs=2))
        dfp = ctx.enter_context(tc.tile_pool(name="dfp", bufs=2))
        rdp = ctx.enter_context(tc.tile_pool(name="rdp", bufs=2))
        esp = ctx.enter_context(tc.tile_pool(name="esp", bufs=3))
        es2p = ctx.enter_context(tc.tile_pool(name="es2p", bufs=2))
        rwp = ctx.enter_context(tc.tile_pool(name="rwp", bufs=2 * NJ))
        ocp = ctx.enter_context(tc.tile_pool(name="ocp", bufs=3))
        stp = ctx.enter_context(tc.tile_pool(name="stp", bufs=1))

        cv = wp.tile([128, 16], dt.float32, tag="cv")
        nc.sync.dma_start(cv[:], CV)
        wk_t = wp.tile([128, NJ * D], dt.bfloat16, tag="wk")
        nc.sync.dma_start(wk_t[:], WK)
        wv_t = wp.tile([128, NJ * D], dt.bfloat16, tag="wv")
        wr_t = wp.tile([128, NJ * D], dt.bfloat16, tag="wr")
        wo_t = wp.tile([128, NJ * D], dt.bfloat16, tag="wo")

        def load_weights_rest():
            nc.sync.dma_start(wr_t[:], WR)
            nc.sync.dma_start(wo_t[:], WO)

        def states(prefix):
            ts_ = []
            for j in range(NJ):
                t = stp.tile([128, 1], dt.float32, tag=f"{prefix}{j}",
                             name=f"{prefix}{j}")
                nc.vector.memset(t[:], 0.0)
                ts_.append(t)
            return ts_

        ast = states("ast")
        bst = states("bst")

        def load_xm(c):
            t0 = c * TC
            xms = {}
            for nm, src in (("xmk", XK), ("xmv", XV), ("xmr", XR)):
                for j in range(NJ):
                    x_ = xmp.tile([128, TC], dt.bfloat16, tag=f"{nm}{j}",
                                  name=f"{nm}{j}")
                    nc.sync.dma_start(
                        x_[:], src[j * 128:(j + 1) * 128, t0:t0 + TC])
                    xms[(nm, j)] = x_
            return xms

        def emit_out_tile(c, rws, i):
            """One output tile (8 accumulating matmuls) of chunk c."""
            t0 = c * TC
            ts_, eh = i // 2, i % 2
            op = ops_.tile([128, 512], dt.float32, tag="op")
            for j in range(NJ):
                nc.tensor.matmul(
                    op[:], rws[j][:, ts_ * 128:(ts_ + 1) * 128],
                    wo_t[:, j * D + eh * 512: j * D + (eh + 1) * 512],
                    start=(j == 0), stop=(j == NJ - 1))
            oc = ocp.tile([128, 512], dt.bfloat16, tag="oc")
            nc.scalar.activation(oc[:], op[:], AF.Copy)
            nc.gpsimd.dma_start(
                O[t0 + ts_ * 128: t0 + (ts_ + 1) * 128,
                  eh * 512:(eh + 1) * 512], oc[:])

        def emit_consume(prev, rws):
            """Scale/combine/divide + gate for channel group e (lagged by 1)."""
            e, ek, a, sa, sb, es = prev
            cc = cv[:, 8 + e:9 + e]
            ac = acp.tile([128, TC], dt.bfloat16, tag="ac")
            nc.scalar.activation(ac[:], a[:], AF.Copy, scale=cc)
            ekc = ekcp.tile([128, TC], dt.bfloat16, tag="ekc")
            nc.scalar.activation(ekc[:], ek[:], AF.Copy, scale=cc)
            num = nump.tile([128, TC], dt.bfloat16, tag="num")
            nc.vector.tensor_tensor(num[:], ac[:], sa[:], OP.add)
            den = denp.tile([128, TC], dt.bfloat16, tag="den")
            nc.vector.tensor_tensor(den[:], ekc[:], sb[:], OP.add)
            es2 = es2p.tile([128, TC], dt.bfloat16, tag="es2")
            nc.vector.tensor_scalar(es2[:], es[:], 1.0, None, OP.add)
            df = dfp.tile([128, TC], dt.float32, tag="df")
            nc.gpsimd.tensor_tensor(df[:], den[:], es2[:], OP.mult)
            rden = rdp.tile([128, TC], dt.float32, tag="rden")
            nc.vector.reciprocal_approx_fast(rden[:], df[:])
            rw = rwp.tile([128, TC], dt.bfloat16, tag="rw")
            nc.gpsimd.tensor_tensor(rw[:], num[:], rden[:], OP.mult)
            rws.append(rw)

        def chunk_phase(c, xms, rws_prev, c_prev):
            """k/v/r matmuls + WKV recurrence + gate; interleaves out tiles
            of chunk c-1. One uniform per-e pipeline, single ACT table."""
            rws = []
            prev = None
            for e in range(NJ):
                kp = kps.tile([128, TC], dt.float32, tag="kp")
                for j in range(NJ):
                    nc.tensor.matmul(
                        kp[:], wk_t[:, j * D + e * 128: j * D + (e + 1) * 128],
                        xms[("xmk", j)][:], start=(j == 0), stop=(j == NJ - 1))
                vp = vps.tile([128, TC], dt.float32, tag="vp")
                for j in range(NJ):
                    nc.tensor.matmul(
                        vp[:], wv_t[:, j * D + e * 128: j * D + (e + 1) * 128],
                        xms[("xmv", j)][:], start=(j == 0), stop=(j == NJ - 1))
                rp = rps.tile([128, TC], dt.float32, tag="rp")
                for j in range(NJ):
                    nc.tensor.matmul(
                        rp[:], wr_t[:, j * D + e * 128: j * D + (e + 1) * 128],
                        xms[("xmr", j)][:], start=(j == 0), stop=(j == NJ - 1))
                if rws_prev is not None and e >= 2:
                    emit_out_tile(c_prev, rws_prev, e - 2)
                ek = ekp.tile([128, TC], dt.bfloat16, tag="ek")
                nc.scalar.activation(ek[:], kp[:], AF.Exp)
                vs = vsp.tile([128, TC], dt.bfloat16, tag="vs")
                nc.scalar.activation(vs[:], vp[:], AF.Copy)
                es = esp.tile([128, TC], dt.bfloat16, tag="es")
                nc.scalar.activation(es[:], rp[:], AF.Exp, scale=-1.0)
                if prev is not None:
                    emit_consume(prev, rws)
                a = ap_.tile([128, TC], dt.bfloat16, tag="a")
                nc.vector.tensor_tensor(a[:], ek[:], vs[:], OP.mult)
                ewb = cv[:, e:e + 1].broadcast_to([128, TC])
                sa = sap.tile([128, TC], dt.bfloat16, tag="sa")
                nc.vector.tensor_tensor_scan(sa[:], ewb, a[:], ast[e][:],
                                             OP.mult, OP.add)
                nc.vector.tensor_copy(ast[e][:], sa[:, TC - 1:TC])
                sb = sbp.tile([128, TC], dt.bfloat16, tag="sb")
                nc.vector.tensor_tensor_scan(sb[:], ewb, ek[:], bst[e][:],
                                             OP.mult, OP.add)
                nc.vector.tensor_copy(bst[e][:], sb[:, TC - 1:TC])
                prev = (e, ek, a, sa, sb, es)
            emit_consume(prev, rws)
            if rws_prev is not None:
                emit_out_tile(c_prev, rws_prev, 6)
                emit_out_tile(c_prev, rws_prev, 7)
            return rws

        # ---- pipelined chunk loop ----
        xms = {}
        for j in range(NJ):
            x_ = xmp.tile([128, TC], dt.bfloat16, tag=f"xmk{j}", name=f"xmk{j}")
            nc.sync.dma_start(x_[:], XK[j * 128:(j + 1) * 128, 0:TC])
            xms[("xmk", j)] = x_
        nc.sync.dma_start(wv_t[:], WV)
        for j in range(NJ):
            x_ = xmp.tile([128, TC], dt.bfloat16, tag=f"xmv{j}", name=f"xmv{j}")
            nc.sync.dma_start(x_[:], XV[j * 128:(j + 1) * 128, 0:TC])
            xms[("xmv", j)] = x_
        for j in range(NJ):
            x_ = xmp.tile([128, TC], dt.bfloat16, tag=f"xmr{j}", name=f"xmr{j}")
            nc.sync.dma_start(x_[:], XR[j * 128:(j + 1) * 128, 0:TC])
            xms[("xmr", j)] = x_
        load_weights_rest()
        rws_prev = None
        for c in range(nch):
            if c + 1 < nch:
                xms_n = load_xm(c + 1)
            rws_prev = chunk_phase(c, xms, rws_prev, c - 1)
            if c + 1 < nch:
                xms = xms_n
        for i in range(2 * NTS):
            emit_out_tile(nch - 1, rws_prev, i)


def pack_inputs(x_slice, time_decay, time_first, time_mix_k, time_mix_v,
                time_mix_r, Wk, Wv, Wr, Wo):
    """Host-side packing for one core. x_slice: [T, D] fp32."""
    import ml_dtypes
    bf16 = ml_dtypes.bfloat16

    def packw(W):
        return np.ascontiguousarray(
            W.T.reshape(NJ, 128, D).transpose(1, 0, 2).reshape(128, NJ * D)
        ).astype(bf16)

    def packw_e(W):
        return np.ascontiguousarray(
            W.reshape(NJ, 128, NJ, 128).transpose(3, 0, 2, 1).reshape(128, NJ * D)
        ).astype(bf16)

    def packv(v):
        return np.ascontiguousarray(v.reshape(NJ, 128).T).astype(np.float32)

    x = np.asarray(x_slice, dtype=np.float32)
    T = x.shape[0]
    xprev = np.zeros_like(x)
    xprev[1:] = x[:-1]

    mk = time_mix_k.reshape(D).astype(np.float32)
    mv = time_mix_v.reshape(D).astype(np.float32)
    mr = time_mix_r.reshape(D).astype(np.float32)

    def mix(m):
        return np.ascontiguousarray((x * m + xprev * (1.0 - m)).T).astype(bf16)

    ew = np.exp(-np.exp(time_decay.astype(np.float64)))
    cc = ew * np.exp(time_first.astype(np.float64)) - 1.0
    cv = np.concatenate(
        [packv(ew.astype(np.float32)), packv(cc.astype(np.float32))],
        axis=1).astype(np.float32)
    return {
        "xmk": mix(mk), "xmv": mix(mv), "xmr": mix(mr),
        "wk": packw(Wk), "wv": packw(Wv), "wr": packw(Wr), "wo": packw(Wo),
        "cv": cv,
    }


# ---------------------------------------------------------------------------
# Harness entry point: full inputs in, full output out, 8-way batch-parallel.
# ---------------------------------------------------------------------------
_CACHE = {}
_last_exec_time_ns = None


def _get_program(n_cores):
    key = ("prog", n_cores)
    if key not in _CACHE:
        nc = bacc.Bacc("TRN2", target_bir_lowering=False, debug=False,
                       num_devices=n_cores)
        build(nc, T=4096)
        nc.compile()
        _CACHE[key] = nc
    return _CACHE[key]


def kernel(x, time_decay, time_first, time_mix_k, time_mix_v, time_mix_r,
           Wk, Wv, Wr, Wo):
    """WKV attention: x [8, 4096, 1024] fp32 -> out [8, 4096, 1024] fp32.

    Shards batch across the 8 NeuronCores (one batch element per core).
    """
    global _last_exec_time_ns
    import os
    from concourse import bass_utils

    x = np.asarray(x, dtype=np.float32)
    B = x.shape[0]
    td = np.asarray(time_decay)
    tf = np.asarray(time_first)
    args = (td, tf, np.asarray(time_mix_k), np.asarray(time_mix_v),
            np.asarray(time_mix_r), np.asarray(Wk), np.asarray(Wv),
            np.asarray(Wr), np.asarray(Wo))
    in_maps = [pack_inputs(x[b], *args) for b in range(B)]

    nc = _get_program(B)
    trace = os.environ.get("WKV_TRACE", "0") == "1"
    r = bass_utils.run_bass_kernel_spmd(nc, in_maps, core_ids=list(range(B)),
                                        trace=trace)
    _last_exec_time_ns = r.exec_time_ns
    return np.stack([r.results[b]["o"] for b in range(B)]).astype(np.float32)


# revision 28
# speedup vs baseline: 1.0151x; 1.0151x over previous
"""RWKV WKV attention kernel for TRN2 (Bass/Tile), batch-parallel over 8 cores.

v3: single fused pass, bf16 matmuls/elementwise, host-precomputed transposed
time-mix inputs (no on-device transposes or mixes), inclusive scans via the
identity num' = a*(ew*e^u - 1) + sa_incl = ew*num (ew cancels in num/den),
ACT-assisted scaling, direct PSUM->DRAM output DMA.

Per core (one batch element, D=1024, T=4096, chunks of TC=512):
  host: xmk/xmv/xmr = time-mixed x, transposed to [D, T], bf16.
  per chunk c, per channel-group e (128 ch):
    k = Wk @ xmk, v = Wv @ xmv (PE, bf16, PSUM fp32)
    ek = exp(k) [ACT], vs = v [ACT copy bf16]
    a = ek*vs [DVE]
    sa = scan(ew, a), sb = scan(ew, ek)  (inclusive, DVE)
    ac = a*c, ekc = ek*c [ACT copy w/ scale], c = ew*e^u - 1
    num = ac + sa, den = ekc + sb [DVE]; rden = 1/den [DVE fp32]
    wkv = num * rden [Pool]
    r = Wr @ xmr (PE); sr = sigmoid(r) [ACT, batched per chunk]
    rw = wkv * sr [Pool]
  out(c) = rw(c)^T @ Wo^T (PE, next-chunk slot), DMA PSUM -> DRAM.

Host-packed weights [128, 8*1024] bf16: arr[p, j*1024+e] = W[e, j*128+p].
cv [128, 16] fp32: col j of each group = channels j*128..j*128+127:
  0-7 ew = exp(-exp(time_decay)), 8-15 c = ew*exp(time_first) - 1.
"""
import sys
for p in ("/opt/trn_rl_repo",):
    if p not in sys.path:
        sys.path.insert(0, p)

import numpy as np
from contextlib import ExitStack

import concourse.bass as bass
import concourse.tile as tile
from concourse import bacc, mybir

dt = mybir.dt
AF = mybir.ActivationFunctionType
OP = mybir.AluOpType

D = 1024
NJ = D // 128  # 8 channel groups


def build(nc, T=4096, TC=512):
    nch = T // TC
    NTS = TC // 128

    XK = nc.dram_tensor("xmk", [D, T], dt.bfloat16, kind="ExternalInput").ap()
    XV = nc.dram_tensor("xmv", [D, T], dt.bfloat16, kind="ExternalInput").ap()
    XR = nc.dram_tensor("xmr", [D, T], dt.bfloat16, kind="ExternalInput").ap()
    WK = nc.dram_tensor("wk", [128, NJ * D], dt.bfloat16, kind="ExternalInput").ap()
    WV = nc.dram_tensor("wv", [128, NJ * D], dt.bfloat16, kind="ExternalInput").ap()
    WR = nc.dram_tensor("wr", [128, NJ * D], dt.bfloat16, kind="ExternalInput").ap()
    WO = nc.dram_tensor("wo", [128, NJ * D], dt.bfloat16, kind="ExternalInput").ap()
    CV = nc.dram_tensor("cv", [128, 16], dt.float32, kind="ExternalInput").ap()
    O = nc.dram_tensor("o", [T, D], dt.bfloat16, kind="ExternalOutput").ap()

    with tile.TileContext(nc) as tc, ExitStack() as ctx:
        wp = ctx.enter_context(tc.tile_pool(name="wp", bufs=1))
        xmp = ctx.enter_context(tc.tile_pool(name="xmp", bufs=2))
        kps = ctx.enter_context(tc.tile_pool(name="kps", bufs=2, space="PSUM"))
        vps = ctx.enter_context(tc.tile_pool(name="vps", bufs=2, space="PSUM"))
        rps = ctx.enter_context(tc.tile_pool(name="rps", bufs=2, space="PSUM"))
        ops_ = ctx.enter_context(tc.tile_pool(name="ops", bufs=2, space="PSUM"))
        ekp = ctx.enter_context(tc.tile_pool(name="ekp", bufs=3))
        vsp = ctx.enter_context(tc.tile_pool(name="vsp", bufs=3))
        ap_ = ctx.enter_context(tc.tile_pool(name="ap", bufs=3))
        sap = ctx.enter_context(tc.tile_pool(name="sap", bufs=3))
        sbp = ctx.enter_context(tc.tile_pool(name="sbp", bufs=3))
        acp = ctx.enter_context(tc.tile_pool(name="acp", bufs=2))
        ekcp = ctx.enter_context(tc.tile_pool(name="ekcp", bufs=2))
        nump = ctx.enter_context(tc.tile_pool(name="nump", bufs=2))
        denp = ctx.enter_context(tc.tile_pool(name="denp", bufs=2))
        dfp = ctx.enter_context(tc.tile_pool(name="dfp", bufs=2))
        rdp = ctx.enter_context(tc.tile_pool(name="rdp", bufs=2))
        esp = ctx.enter_context(tc.tile_pool(name="esp", bufs=3))
        es2p = ctx.enter_context(tc.tile_pool(name="es2p", bufs=2))
        rwp = ctx.enter_context(tc.tile_pool(name="rwp", bufs=2 * NJ))
        ocp = ctx.enter_context(tc.tile_pool(name="ocp", bufs=3))
        stp = ctx.enter_context(tc.tile_pool(name="stp", bufs=1))

        cv = wp.tile([128, 16], dt.float32, tag="cv")
        nc.sync.dma_start(cv[:], CV)
        wk_t = wp.tile([128, NJ * D], dt.bfloat16, tag="wk")
        nc.sync.dma_start(wk_t[:], WK)
        wv_t = wp.tile([128, NJ * D], dt.bfloat16, tag="wv")
        wr_t = wp.tile([128, NJ * D], dt.bfloat16, tag="wr")
        wo_t = wp.tile([128, NJ * D], dt.bfloat16, tag="wo")

        def load_weights_rest():
            nc.sync.dma_start(wr_t[:], WR)
            nc.sync.dma_start(wo_t[:], WO)

        def states(prefix):
            ts_ = []
            for j in range(NJ):
                t = stp.tile([128, 1], dt.float32, tag=f"{prefix}{j}",
                             name=f"{prefix}{j}")
                nc.vector.memset(t[:], 0.0)
                ts_.append(t)
            return ts_

        ast = states("ast")
        bst = states("bst")

        def load_xm(c):
            t0 = c * TC
            xms = {}
            for nm, src in (("xmk", XK), ("xmv", XV), ("xmr", XR)):
                for j in range(NJ):
                    x_ = xmp.tile([128, TC], dt.bfloat16, tag=f"{nm}{j}",
                                  name=f"{nm}{j}")
                    nc.sync.dma_start(
                        x_[:], src[j * 128:(j + 1) * 128, t0:t0 + TC])
                    xms[(nm, j)] = x_
            return xms

        def emit_out_tile(c, rws, i):
            """One output tile (8 accumulating matmuls) of chunk c."""
            t0 = c * TC
            ts_, eh = i // 2, i % 2
            op = ops_.tile([128, 512], dt.float32, tag="op")
            for j in range(NJ):
                nc.tensor.matmul(
                    op[:], rws[j][:, ts_ * 128:(ts_ + 1) * 128],
                    wo_t[:, j * D + eh * 512: j * D + (eh + 1) * 512],
                    start=(j == 0), stop=(j == NJ - 1))
            oc = ocp.tile([128, 512], dt.bfloat16, tag="oc")
            nc.scalar.activation(oc[:], op[:], AF.Copy)
            nc.gpsimd.dma_start(
                O[t0 + ts_ * 128: t0 + (ts_ + 1) * 128,
                  eh * 512:(eh + 1) * 512], oc[:])

        def emit_consume(prev, rws):
            """Scale/combine/divide + gate for channel group e (lagged by 1)."""
            e, ek, a, sa, sb, es = prev
            cc = cv[:, 8 + e:9 + e]
            ac = acp.tile([128, TC], dt.bfloat16, tag="ac")
            nc.scalar.activation(ac[:], a[:], AF.Copy, scale=cc)
            ekc = ekcp.tile([128, TC], dt.bfloat16, tag="ekc")
            nc.scalar.activation(ekc[:], ek[:], AF.Copy, scale=cc)
            num = nump.tile([128, TC], dt.bfloat16, tag="num")
            nc.vector.tensor_tensor(num[:], ac[:], sa[:], OP.add)
            den = denp.tile([128, TC], dt.bfloat16, tag="den")
            nc.vector.tensor_tensor(den[:], ekc[:], sb[:], OP.add)
            es2 = es2p.tile([128, TC], dt.bfloat16, tag="es2")
            nc.vector.tensor_scalar(es2[:], es[:], 1.0, None, OP.add)
            df = dfp.tile([128, TC], dt.float32, tag="df")
            nc.gpsimd.tensor_tensor(df[:], den[:], es2[:], OP.mult)
            rden = rdp.tile([128, TC], dt.float32, tag="rden")
            nc.vector.reciprocal_approx_fast(rden[:], df[:])
            rw = rwp.tile([128, TC], dt.bfloat16, tag="rw")
            nc.gpsimd.tensor_tensor(rw[:], num[:], rden[:], OP.mult)
            rws.append(rw)

        def chunk_phase(c, xms, rws_prev, c_prev):
            """k/v/r matmuls + WKV recurrence + gate; interleaves out tiles
            of chunk c-1. One uniform per-e pipeline, single ACT table."""
            rws = []
            prev = None
            for e in range(NJ):
                kp = kps.tile([128, TC], dt.float32, tag="kp")
                for j in range(NJ):
                    nc.tensor.matmul(
                        kp[:], wk_t[:, j * D + e * 128: j * D + (e + 1) * 128],
                        xms[("xmk", j)][:], start=(j == 0), stop=(j == NJ - 1))
                vp = vps.tile([128, TC], dt.float32, tag="vp")
                for j in range(NJ):
                    nc.tensor.matmul(
                        vp[:], wv_t[:, j * D + e * 128: j * D + (e + 1) * 128],
                        xms[("xmv", j)][:], start=(j == 0), stop=(j == NJ - 1))
                rp = rps.tile([128, TC], dt.float32, tag="rp")
                for j in range(NJ):
                    nc.tensor.matmul(
                        rp[:], wr_t[:, j * D + e * 128: j * D + (e + 1) * 128],
                        xms[("xmr", j)][:], start=(j == 0), stop=(j == NJ - 1))
                if rws_prev is not None:
                    emit_out_tile(c_prev, rws_prev, e)
                ek = ekp.tile([128, TC], dt.bfloat16, tag="ek")
                nc.scalar.activation(ek[:], kp[:], AF.Exp)
                vs = vsp.tile([128, TC], dt.bfloat16, tag="vs")
                nc.scalar.activation(vs[:], vp[:], AF.Copy)
                es = esp.tile([128, TC], dt.bfloat16, tag="es")
                nc.scalar.activation(es[:], rp[:], AF.Exp, scale=-1.0)
                if prev is not None:
                    emit_consume(prev, rws)
                a = ap_.tile([128, TC], dt.bfloat16, tag="a")
                nc.vector.tensor_tensor(a[:], ek[:], vs[:], OP.mult)
                ewb = cv[:, e:e + 1].broadcast_to([128, TC])
                sa = sap.tile([128, TC], dt.bfloat16, tag="sa")
                nc.vector.tensor_tensor_scan(sa[:], ewb, a[:], ast[e][:],
                                             OP.mult, OP.add)
                nc.vector.tensor_copy(ast[e][:], sa[:, TC - 1:TC])
                sb = sbp.tile([128, TC], dt.bfloat16, tag="sb")
                nc.vector.tensor_tensor_scan(sb[:], ewb, ek[:], bst[e][:],
                                             OP.mult, OP.add)
                nc.vector.tensor_copy(bst[e][:], sb[:, TC - 1:TC])
                prev = (e, ek, a, sa, sb, es)
            emit_consume(prev, rws)
            return rws

        # ---- pipelined chunk loop ----
        xms = {}
        for j in range(NJ):
            x_ = xmp.tile([128, TC], dt.bfloat16, tag=f"xmk{j}", name=f"xmk{j}")
            nc.sync.dma_start(x_[:], XK[j * 128:(j + 1) * 128, 0:TC])
            xms[("xmk", j)] = x_
        nc.sync.dma_start(wv_t[:], WV)
        for j in range(NJ):
            x_ = xmp.tile([128, TC], dt.bfloat16, tag=f"xmv{j}", name=f"xmv{j}")
            nc.sync.dma_start(x_[:], XV[j * 128:(j + 1) * 128, 0:TC])
            xms[("xmv", j)] = x_
        for j in range(NJ):
            x_ = xmp.tile([128, TC], dt.bfloat16, tag=f"xmr{j}", name=f"xmr{j}")
            nc.sync.dma_start(x_[:], XR[j * 128:(j + 1) * 128, 0:TC])
            xms[("xmr", j)] = x_
        load_weights_rest()
        rws_prev = None
        for c in range(nch):
            if c + 1 < nch:
                xms_n = load_xm(c + 1)
            rws_prev = chunk_phase(c, xms, rws_prev, c - 1)
            if c + 1 < nch:
                xms = xms_n
        for i in range(2 * NTS):
            emit_out_tile(nch - 1, rws_prev, i)


def pack_inputs(x_slice, time_decay, time_first, time_mix_k, time_mix_v,
                time_mix_r, Wk, Wv, Wr, Wo):
    """Host-side packing for one core. x_slice: [T, D] fp32."""
    import ml_dtypes
    bf16 = ml_dtypes.bfloat16

    def packw(W):
        return np.ascontiguousarray(
            W.T.reshape(NJ, 128, D).transpose(1, 0, 2).reshape(128, NJ * D)
        ).astype(bf16)

    def packw_e(W):
        return np.ascontiguousarray(
            W.reshape(NJ, 128, NJ, 128).transpose(3, 0, 2, 1).reshape(128, NJ * D)
        ).astype(bf16)

    def packv(v):
        return np.ascontiguousarray(v.reshape(NJ, 128).T).astype(np.float32)

    x = np.asarray(x_slice, dtype=np.float32)
    T = x.shape[0]
    xprev = np.zeros_like(x)
    xprev[1:] = x[:-1]

    mk = time_mix_k.reshape(D).astype(np.float32)
    mv = time_mix_v.reshape(D).astype(np.float32)
    mr = time_mix_r.reshape(D).astype(np.float32)

    def mix(m):
        return np.ascontiguousarray((x * m + xprev * (1.0 - m)).T).astype(bf16)

    ew = np.exp(-np.exp(time_decay.astype(np.float64)))
    cc = ew * np.exp(time_first.astype(np.float64)) - 1.0
    cv = np.concatenate(
        [packv(ew.astype(np.float32)), packv(cc.astype(np.float32))],
        axis=1).astype(np.float32)
    return {
        "xmk": mix(mk), "xmv": mix(mv), "xmr": mix(mr),
        "wk": packw(Wk), "wv": packw(Wv), "wr": packw(Wr), "wo": packw(Wo),
        "cv": cv,
    }


# ---------------------------------------------------------------------------
# Harness entry point: full inputs in, full output out, 8-way batch-parallel.
# ---------------------------------------------------------------------------
_CACHE = {}
_last_exec_time_ns = None


def _get_program(n_cores):
    key = ("prog", n_cores)
    if key not in _CACHE:
        nc = bacc.Bacc("TRN2", target_bir_lowering=False, debug=False,
                       num_devices=n_cores)
        build(nc, T=4096)
        nc.compile()
        _CACHE[key] = nc
    return _CACHE[key]


def kernel(x, time_decay, time_first, time_mix_k, time_mix_v, time_mix_r,
           Wk, Wv, Wr, Wo):
    """WKV attention: x [8, 4096, 1024] fp32 -> out [8, 4096, 1024] fp32.

    Shards batch across the 8 NeuronCores (one batch element per core).
    """
    global _last_exec_time_ns
    import os
    from concourse import bass_utils

    x = np.asarray(x, dtype=np.float32)
    B = x.shape[0]
    td = np.asarray(time_decay)
    tf = np.asarray(time_first)
    args = (td, tf, np.asarray(time_mix_k), np.asarray(time_mix_v),
            np.asarray(time_mix_r), np.asarray(Wk), np.asarray(Wv),
            np.asarray(Wr), np.asarray(Wo))
    in_maps = [pack_inputs(x[b], *args) for b in range(B)]

    nc = _get_program(B)
    trace = os.environ.get("WKV_TRACE", "0") == "1"
    r = bass_utils.run_bass_kernel_spmd(nc, in_maps, core_ids=list(range(B)),
                                        trace=trace)
    _last_exec_time_ns = r.exec_time_ns
    return np.stack([r.results[b]["o"] for b in range(B)]).astype(np.float32)


# revision 29
# speedup vs baseline: 1.0822x; 1.0661x over previous
"""RWKV WKV attention kernel for TRN2 (Bass/Tile), batch-parallel over 8 cores.

v3: single fused pass, bf16 matmuls/elementwise, host-precomputed transposed
time-mix inputs (no on-device transposes or mixes), inclusive scans via the
identity num' = a*(ew*e^u - 1) + sa_incl = ew*num (ew cancels in num/den),
ACT-assisted scaling, direct PSUM->DRAM output DMA.

Per core (one batch element, D=1024, T=4096, chunks of TC=512):
  host: xmk/xmv/xmr = time-mixed x, transposed to [D, T], bf16.
  per chunk c, per channel-group e (128 ch):
    k = Wk @ xmk, v = Wv @ xmv (PE, bf16, PSUM fp32)
    ek = exp(k) [ACT], vs = v [ACT copy bf16]
    a = ek*vs [DVE]
    sa = scan(ew, a), sb = scan(ew, ek)  (inclusive, DVE)
    ac = a*c, ekc = ek*c [ACT copy w/ scale], c = ew*e^u - 1
    num = ac + sa, den = ekc + sb [DVE]; rden = 1/den [DVE fp32]
    wkv = num * rden [Pool]
    r = Wr @ xmr (PE); sr = sigmoid(r) [ACT, batched per chunk]
    rw = wkv * sr [Pool]
  out(c) = rw(c)^T @ Wo^T (PE, next-chunk slot), DMA PSUM -> DRAM.

Host-packed weights [128, 8*1024] bf16: arr[p, j*1024+e] = W[e, j*128+p].
cv [128, 16] fp32: col j of each group = channels j*128..j*128+127:
  0-7 ew = exp(-exp(time_decay)), 8-15 c = ew*exp(time_first) - 1.
"""
import sys
for p in ("/opt/trn_rl_repo",):
    if p not in sys.path:
        sys.path.insert(0, p)

import numpy as np
from contextlib import ExitStack

import concourse.bass as bass
import concourse.tile as tile
from concourse import bacc, mybir

dt = mybir.dt
AF = mybir.ActivationFunctionType
OP = mybir.AluOpType

D = 1024
NJ = D // 128  # 8 channel groups


def build(nc, T=4096, TC=512):
    nch = T // TC
    NTS = TC // 128

    XK = nc.dram_tensor("xmk", [D, T], dt.bfloat16, kind="ExternalInput").ap()
    XV = nc.dram_tensor("xmv", [D, T], dt.bfloat16, kind="ExternalInput").ap()
    XR = nc.dram_tensor("xmr", [D // 2, 2, T], dt.float8e4,
                        kind="ExternalInput").ap()
    WK = nc.dram_tensor("wk", [128, NJ * D], dt.bfloat16, kind="ExternalInput").ap()
    WV = nc.dram_tensor("wv", [128, NJ * D], dt.bfloat16, kind="ExternalInput").ap()
    WR = nc.dram_tensor("wr", [128, NJ * NJ // 2, 2, 128], dt.float8e4,
                        kind="ExternalInput").ap()
    WO = nc.dram_tensor("wo", [128, NJ * D], dt.bfloat16, kind="ExternalInput").ap()
    CV = nc.dram_tensor("cv", [128, 16], dt.float32, kind="ExternalInput").ap()
    O = nc.dram_tensor("o", [T, D], dt.bfloat16, kind="ExternalOutput").ap()

    with tile.TileContext(nc) as tc, ExitStack() as ctx:
        wp = ctx.enter_context(tc.tile_pool(name="wp", bufs=1))
        xmp = ctx.enter_context(tc.tile_pool(name="xmp", bufs=2))
        kps = ctx.enter_context(tc.tile_pool(name="kps", bufs=2, space="PSUM"))
        vps = ctx.enter_context(tc.tile_pool(name="vps", bufs=2, space="PSUM"))
        rps = ctx.enter_context(tc.tile_pool(name="rps", bufs=2, space="PSUM"))
        ops_ = ctx.enter_context(tc.tile_pool(name="ops", bufs=2, space="PSUM"))
        ekp = ctx.enter_context(tc.tile_pool(name="ekp", bufs=3))
        vsp = ctx.enter_context(tc.tile_pool(name="vsp", bufs=3))
        ap_ = ctx.enter_context(tc.tile_pool(name="ap", bufs=3))
        sap = ctx.enter_context(tc.tile_pool(name="sap", bufs=3))
        sbp = ctx.enter_context(tc.tile_pool(name="sbp", bufs=3))
        acp = ctx.enter_context(tc.tile_pool(name="acp", bufs=2))
        ekcp = ctx.enter_context(tc.tile_pool(name="ekcp", bufs=2))
        nump = ctx.enter_context(tc.tile_pool(name="nump", bufs=2))
        denp = ctx.enter_context(tc.tile_pool(name="denp", bufs=2))
        dfp = ctx.enter_context(tc.tile_pool(name="dfp", bufs=2))
        rdp = ctx.enter_context(tc.tile_pool(name="rdp", bufs=2))
        esp = ctx.enter_context(tc.tile_pool(name="esp", bufs=3))
        es2p = ctx.enter_context(tc.tile_pool(name="es2p", bufs=2))
        rwp = ctx.enter_context(tc.tile_pool(name="rwp", bufs=2 * NJ))
        ocp = ctx.enter_context(tc.tile_pool(name="ocp", bufs=3))
        stp = ctx.enter_context(tc.tile_pool(name="stp", bufs=1))

        cv = wp.tile([128, 16], dt.float32, tag="cv")
        nc.sync.dma_start(cv[:], CV)
        wk_t = wp.tile([128, NJ * D], dt.bfloat16, tag="wk")
        nc.sync.dma_start(wk_t[:], WK)
        wv_t = wp.tile([128, NJ * D], dt.bfloat16, tag="wv")
        wr_t = wp.tile([128, NJ * NJ // 2, 2, 128], dt.float8e4, tag="wr")
        wo_t = wp.tile([128, NJ * D], dt.bfloat16, tag="wo")

        def load_weights_rest():
            nc.sync.dma_start(wr_t[:], WR)
            nc.sync.dma_start(wo_t[:], WO)

        def states(prefix):
            ts_ = []
            for j in range(NJ):
                t = stp.tile([128, 1], dt.float32, tag=f"{prefix}{j}",
                             name=f"{prefix}{j}")
                nc.vector.memset(t[:], 0.0)
                ts_.append(t)
            return ts_

        ast = states("ast")
        bst = states("bst")

        def load_xm(c):
            t0 = c * TC
            xms = {}
            for nm, src in (("xmk", XK), ("xmv", XV)):
                for j in range(NJ):
                    x_ = xmp.tile([128, TC], dt.bfloat16, tag=f"{nm}{j}",
                                  name=f"{nm}{j}")
                    nc.sync.dma_start(
                        x_[:], src[j * 128:(j + 1) * 128, t0:t0 + TC])
                    xms[(nm, j)] = x_
            for q in range(NJ // 2):
                x_ = xmp.tile([128, 2, TC], dt.float8e4, tag=f"xmr{q}",
                              name=f"xmr{q}")
                nc.sync.dma_start(
                    x_[:], XR[q * 128:(q + 1) * 128, :, t0:t0 + TC])
                xms[("xmr", q)] = x_
            return xms

        def emit_out_tile(c, rws, i):
            """One output tile (8 accumulating matmuls) of chunk c."""
            t0 = c * TC
            ts_, eh = i // 2, i % 2
            op = ops_.tile([128, 512], dt.float32, tag="op")
            for j in range(NJ):
                nc.tensor.matmul(
                    op[:], rws[j][:, ts_ * 128:(ts_ + 1) * 128],
                    wo_t[:, j * D + eh * 512: j * D + (eh + 1) * 512],
                    start=(j == 0), stop=(j == NJ - 1))
            oc = ocp.tile([128, 512], dt.bfloat16, tag="oc")
            nc.scalar.activation(oc[:], op[:], AF.Copy)
            nc.gpsimd.dma_start(
                O[t0 + ts_ * 128: t0 + (ts_ + 1) * 128,
                  eh * 512:(eh + 1) * 512], oc[:])

        def emit_consume(prev, rws):
            """Scale/combine/divide + gate for channel group e (lagged by 1)."""
            e, ek, a, sa, sb, es = prev
            cc = cv[:, 8 + e:9 + e]
            ac = acp.tile([128, TC], dt.bfloat16, tag="ac")
            nc.scalar.activation(ac[:], a[:], AF.Copy, scale=cc)
            ekc = ekcp.tile([128, TC], dt.bfloat16, tag="ekc")
            nc.scalar.activation(ekc[:], ek[:], AF.Copy, scale=cc)
            num = nump.tile([128, TC], dt.bfloat16, tag="num")
            nc.vector.tensor_tensor(num[:], ac[:], sa[:], OP.add)
            den = denp.tile([128, TC], dt.bfloat16, tag="den")
            nc.vector.tensor_tensor(den[:], ekc[:], sb[:], OP.add)
            es2 = es2p.tile([128, TC], dt.bfloat16, tag="es2")
            nc.vector.tensor_scalar(es2[:], es[:], 1.0, None, OP.add)
            df = dfp.tile([128, TC], dt.float32, tag="df")
            nc.gpsimd.tensor_tensor(df[:], den[:], es2[:], OP.mult)
            rden = rdp.tile([128, TC], dt.float32, tag="rden")
            nc.vector.reciprocal_approx_fast(rden[:], df[:])
            rw = rwp.tile([128, TC], dt.bfloat16, tag="rw")
            nc.gpsimd.tensor_tensor(rw[:], num[:], rden[:], OP.mult)
            rws.append(rw)

        def chunk_phase(c, xms, rws_prev, c_prev):
            """k/v/r matmuls + WKV recurrence + gate; interleaves out tiles
            of chunk c-1. One uniform per-e pipeline, single ACT table."""
            rws = []
            prev = None
            for e in range(NJ):
                kp = kps.tile([128, TC], dt.float32, tag="kp")
                for j in range(NJ):
                    nc.tensor.matmul(
                        kp[:], wk_t[:, j * D + e * 128: j * D + (e + 1) * 128],
                        xms[("xmk", j)][:], start=(j == 0), stop=(j == NJ - 1))
                vp = vps.tile([128, TC], dt.float32, tag="vp")
                for j in range(NJ):
                    nc.tensor.matmul(
                        vp[:], wv_t[:, j * D + e * 128: j * D + (e + 1) * 128],
                        xms[("xmv", j)][:], start=(j == 0), stop=(j == NJ - 1))
                rp = rps.tile([128, TC], dt.float32, tag="rp")
                for q in range(NJ // 2):
                    nc.tensor.matmul(
                        rp[:], wr_t[:, q * NJ + e, :, :], xms[("xmr", q)][:],
                        start=(q == 0), stop=(q == NJ // 2 - 1),
                        perf_mode=mybir.MatmulPerfMode.DoubleRow)
                if rws_prev is not None:
                    emit_out_tile(c_prev, rws_prev, e)
                ek = ekp.tile([128, TC], dt.bfloat16, tag="ek")
                nc.scalar.activation(ek[:], kp[:], AF.Exp)
                vs = vsp.tile([128, TC], dt.bfloat16, tag="vs")
                nc.scalar.activation(vs[:], vp[:], AF.Copy)
                es = esp.tile([128, TC], dt.bfloat16, tag="es")
                nc.scalar.activation(es[:], rp[:], AF.Exp, scale=-1.0)
                if prev is not None:
                    emit_consume(prev, rws)
                a = ap_.tile([128, TC], dt.bfloat16, tag="a")
                nc.vector.tensor_tensor(a[:], ek[:], vs[:], OP.mult)
                ewb = cv[:, e:e + 1].broadcast_to([128, TC])
                sa = sap.tile([128, TC], dt.bfloat16, tag="sa")
                nc.vector.tensor_tensor_scan(sa[:], ewb, a[:], ast[e][:],
                                             OP.mult, OP.add)
                nc.vector.tensor_copy(ast[e][:], sa[:, TC - 1:TC])
                sb = sbp.tile([128, TC], dt.bfloat16, tag="sb")
                nc.vector.tensor_tensor_scan(sb[:], ewb, ek[:], bst[e][:],
                                             OP.mult, OP.add)
                nc.vector.tensor_copy(bst[e][:], sb[:, TC - 1:TC])
                prev = (e, ek, a, sa, sb, es)
            emit_consume(prev, rws)
            return rws

        # ---- pipelined chunk loop ----
        xms = {}
        for j in range(NJ):
            x_ = xmp.tile([128, TC], dt.bfloat16, tag=f"xmk{j}", name=f"xmk{j}")
            nc.sync.dma_start(x_[:], XK[j * 128:(j + 1) * 128, 0:TC])
            xms[("xmk", j)] = x_
        nc.sync.dma_start(wv_t[:], WV)
        for j in range(NJ):
            x_ = xmp.tile([128, TC], dt.bfloat16, tag=f"xmv{j}", name=f"xmv{j}")
            nc.sync.dma_start(x_[:], XV[j * 128:(j + 1) * 128, 0:TC])
            xms[("xmv", j)] = x_
        for q in range(NJ // 2):
            x_ = xmp.tile([128, 2, TC], dt.float8e4, tag=f"xmr{q}", name=f"xmr{q}")
            nc.sync.dma_start(x_[:], XR[q * 128:(q + 1) * 128, :, 0:TC])
            xms[("xmr", q)] = x_
        load_weights_rest()
        rws_prev = None
        for c in range(nch):
            if c + 1 < nch:
                xms_n = load_xm(c + 1)
            rws_prev = chunk_phase(c, xms, rws_prev, c - 1)
            if c + 1 < nch:
                xms = xms_n
        for i in range(2 * NTS):
            emit_out_tile(nch - 1, rws_prev, i)


def pack_inputs(x_slice, time_decay, time_first, time_mix_k, time_mix_v,
                time_mix_r, Wk, Wv, Wr, Wo):
    """Host-side packing for one core. x_slice: [T, D] fp32."""
    import ml_dtypes
    bf16 = ml_dtypes.bfloat16

    def packw(W):
        return np.ascontiguousarray(
            W.T.reshape(NJ, 128, D).transpose(1, 0, 2).reshape(128, NJ * D)
        ).astype(bf16)

    def packw_e(W):
        return np.ascontiguousarray(
            W.reshape(NJ, 128, NJ, 128).transpose(3, 0, 2, 1).reshape(128, NJ * D)
        ).astype(bf16)

    def packv(v):
        return np.ascontiguousarray(v.reshape(NJ, 128).T).astype(np.float32)

    x = np.asarray(x_slice, dtype=np.float32)
    T = x.shape[0]
    xprev = np.zeros_like(x)
    xprev[1:] = x[:-1]

    mk = time_mix_k.reshape(D).astype(np.float32)
    mv = time_mix_v.reshape(D).astype(np.float32)
    mr = time_mix_r.reshape(D).astype(np.float32)

    def mix(m):
        return np.ascontiguousarray((x * m + xprev * (1.0 - m)).T).astype(bf16)

    fp8 = ml_dtypes.float8_e4m3

    def mix8(m):
        xm = (x * m + xprev * (1.0 - m)).T  # [D, T] fp32
        T_ = xm.shape[1]
        return np.ascontiguousarray(
            xm.reshape(NJ // 2, 2, 128, T_).transpose(0, 2, 1, 3)
            .reshape(D // 2, 2, T_)).astype(fp8)

    def packw8(W):
        r = W.reshape(NJ, 128, NJ // 2, 2, 128)  # [e, m, q, i, p]
        return np.ascontiguousarray(
            r.transpose(4, 2, 0, 3, 1).reshape(128, NJ * NJ // 2, 2, 128)
        ).astype(fp8)

    ew = np.exp(-np.exp(time_decay.astype(np.float64)))
    cc = ew * np.exp(time_first.astype(np.float64)) - 1.0
    cv = np.concatenate(
        [packv(ew.astype(np.float32)), packv(cc.astype(np.float32))],
        axis=1).astype(np.float32)
    return {
        "xmk": mix(mk), "xmv": mix(mv), "xmr": mix8(mr),
        "wk": packw(Wk), "wv": packw(Wv), "wr": packw8(Wr), "wo": packw(Wo),
        "cv": cv,
    }


# ---------------------------------------------------------------------------
# Harness entry point: full inputs in, full output out, 8-way batch-parallel.
# ---------------------------------------------------------------------------
_CACHE = {}
_last_exec_time_ns = None


def _get_program(n_cores):
    key = ("prog", n_cores)
    if key not in _CACHE:
        nc = bacc.Bacc("TRN2", target_bir_lowering=False, debug=False,
                       num_devices=n_cores)
        build(nc, T=4096)
        nc.compile()
        _CACHE[key] = nc
    return _CACHE[key]


def kernel(x, time_decay, time_first, time_mix_k, time_mix_v, time_mix_r,
           Wk, Wv, Wr, Wo):
    """WKV attention: x [8, 4096, 1024] fp32 -> out [8, 4096, 1024] fp32.

    Shards batch across the 8 NeuronCores (one batch element per core).
    """
    global _last_exec_time_ns
    import os
    from concourse import bass_utils

    x = np.asarray(x, dtype=np.float32)
    B = x.shape[0]
    td = np.asarray(time_decay)
    tf = np.asarray(time_first)
    args = (td, tf, np.asarray(time_mix_k), np.asarray(time_mix_v),
            np.asarray(time_mix_r), np.asarray(Wk), np.asarray(Wv),
            np.asarray(Wr), np.asarray(Wo))
    in_maps = [pack_inputs(x[b], *args) for b in range(B)]

    nc = _get_program(B)
    trace = os.environ.get("WKV_TRACE", "0") == "1"
    r = bass_utils.run_bass_kernel_spmd(nc, in_maps, core_ids=list(range(B)),
                                        trace=trace)
    _last_exec_time_ns = r.exec_time_ns
    return np.stack([r.results[b]["o"] for b in range(B)]).astype(np.float32)


# revision 33
# speedup vs baseline: 1.0893x; 1.0066x over previous
"""RWKV WKV attention kernel for TRN2 (Bass/Tile), batch-parallel over 8 cores.

Single fused pass per chunk (TC=512, 8 chunks), one uniform per-e pipeline:
  k = Wk@xmk, v = Wv@xmv (PE bf16), r = Wr@xmr (PE fp8-e4m3 DoubleRow)
  ek = exp(k), vs = v, es = exp(-r)   [ACT, single Exp table, no switches]
  a = ek*vs;  sa = scan(ew, a), sb = scan(ew, ek)   [DVE, inclusive scans]
  num = a*c + sa, den = ek*c + sb  with c = ew*e^u - 1
    (inclusive-scan identity: num = ew*(alpha_{t-1} + e^u a_t); the ew
     factor cancels in the ratio, so no shifted/halo views are needed)
  rw = num / (den * (1 + es))          [sigmoid folded into the division]
  out tiles of chunk c-1 interleave into chunk c's PE stream (slots 2..7),
  trailing two ride 2 chunks behind; out = rw^T @ Wo^T, PSUM -> bf16 -> DRAM.

Host precomputes the three time-mixed inputs transposed to [D, T] (bf16;
xmr packed fp8 as [D/2, 2, T] q-pair blocks for DoubleRow), the packed
weights, and cv [128,16] fp32 (ew groups in cols 0-7, c groups in 8-15).
Output DRAM is bf16; host upcasts to fp32. rel_err ~1.5e-2 (gate 2e-2).

Learned the hard way (do not "fix"):
  - weight tiles must each be written by exactly ONE DMA, j-major packing;
    multi-DMA or per-e weight tiles serialize LDWEIGHTS (+25% PE time)
  - PSUM drains (ek/vs/es/oc) must stay on ACT; moving any to DVE stalls PE
  - run-to-run variance is bimodal (~492 vs ~590 on the pre-fp8 build);
    judge changes with >=2 runs
"""
import sys
for p in ("/opt/trn_rl_repo",):
    if p not in sys.path:
        sys.path.insert(0, p)

import numpy as np
from contextlib import ExitStack

import concourse.bass as bass
import concourse.tile as tile
from concourse import bacc, mybir

dt = mybir.dt
AF = mybir.ActivationFunctionType
OP = mybir.AluOpType

D = 1024
NJ = D // 128  # 8 channel groups


def build(nc, T=4096, TC=512):
    nch = T // TC
    NTS = TC // 128

    XK = nc.dram_tensor("xmk", [D, T], dt.bfloat16, kind="ExternalInput").ap()
    XV = nc.dram_tensor("xmv", [D, T], dt.bfloat16, kind="ExternalInput").ap()
    XR = nc.dram_tensor("xmr", [D // 2, 2, T], dt.float8e4,
                        kind="ExternalInput").ap()
    WK = nc.dram_tensor("wk", [128, NJ * D], dt.bfloat16, kind="ExternalInput").ap()
    WV = nc.dram_tensor("wv", [128, NJ * D], dt.bfloat16, kind="ExternalInput").ap()
    WR = nc.dram_tensor("wr", [128, NJ * NJ // 2, 2, 128], dt.float8e4,
                        kind="ExternalInput").ap()
    WO = nc.dram_tensor("wo", [128, NJ * D], dt.bfloat16, kind="ExternalInput").ap()
    CV = nc.dram_tensor("cv", [128, 16], dt.float32, kind="ExternalInput").ap()
    O = nc.dram_tensor("o", [T, D], dt.bfloat16, kind="ExternalOutput").ap()

    with tile.TileContext(nc) as tc, ExitStack() as ctx:
        wp = ctx.enter_context(tc.tile_pool(name="wp", bufs=1))
        xmp = ctx.enter_context(tc.tile_pool(name="xmp", bufs=2))
        kps = ctx.enter_context(tc.tile_pool(name="kps", bufs=2, space="PSUM"))
        vps = ctx.enter_context(tc.tile_pool(name="vps", bufs=2, space="PSUM"))
        rps = ctx.enter_context(tc.tile_pool(name="rps", bufs=2, space="PSUM"))
        ops_ = ctx.enter_context(tc.tile_pool(name="ops", bufs=2, space="PSUM"))
        ekp = ctx.enter_context(tc.tile_pool(name="ekp", bufs=3))
        vsp = ctx.enter_context(tc.tile_pool(name="vsp", bufs=3))
        ap_ = ctx.enter_context(tc.tile_pool(name="ap", bufs=3))
        sap = ctx.enter_context(tc.tile_pool(name="sap", bufs=3))
        sbp = ctx.enter_context(tc.tile_pool(name="sbp", bufs=3))
        acp = ctx.enter_context(tc.tile_pool(name="acp", bufs=2))
        ekcp = ctx.enter_context(tc.tile_pool(name="ekcp", bufs=2))
        nump = ctx.enter_context(tc.tile_pool(name="nump", bufs=2))
        denp = ctx.enter_context(tc.tile_pool(name="denp", bufs=2))
        dfp = ctx.enter_context(tc.tile_pool(name="dfp", bufs=2))
        rdp = ctx.enter_context(tc.tile_pool(name="rdp", bufs=2))
        esp = ctx.enter_context(tc.tile_pool(name="esp", bufs=3))
        es2p = ctx.enter_context(tc.tile_pool(name="es2p", bufs=2))
        rwp = ctx.enter_context(tc.tile_pool(name="rwp", bufs=2 * NJ))
        ocp = ctx.enter_context(tc.tile_pool(name="ocp", bufs=3))
        stp = ctx.enter_context(tc.tile_pool(name="stp", bufs=1))

        cv = wp.tile([128, 16], dt.float32, tag="cv")
        nc.sync.dma_start(cv[:], CV)
        wk_t = wp.tile([128, NJ * D], dt.bfloat16, tag="wk")
        nc.sync.dma_start(wk_t[:], WK)
        wv_t = wp.tile([128, NJ * D], dt.bfloat16, tag="wv")
        wr_t = wp.tile([128, NJ * NJ // 2, 2, 128], dt.float8e4, tag="wr")
        wo_t = wp.tile([128, NJ * D], dt.bfloat16, tag="wo")

        def load_weights_rest():
            nc.sync.dma_start(wr_t[:], WR)
            nc.sync.dma_start(wo_t[:], WO)

        def states(prefix):
            ts_ = []
            for j in range(NJ):
                t = stp.tile([128, 1], dt.float32, tag=f"{prefix}{j}",
                             name=f"{prefix}{j}")
                nc.vector.memset(t[:], 0.0)
                ts_.append(t)
            return ts_

        ast = states("ast")
        bst = states("bst")

        def load_xm(c):
            t0 = c * TC
            xms = {}
            for nm, src in (("xmk", XK), ("xmv", XV)):
                for j in range(NJ):
                    x_ = xmp.tile([128, TC], dt.bfloat16, tag=f"{nm}{j}",
                                  name=f"{nm}{j}")
                    nc.sync.dma_start(
                        x_[:], src[j * 128:(j + 1) * 128, t0:t0 + TC])
                    xms[(nm, j)] = x_
            for q in range(NJ // 2):
                x_ = xmp.tile([128, 2, TC], dt.float8e4, tag=f"xmr{q}",
                              name=f"xmr{q}")
                nc.sync.dma_start(
                    x_[:], XR[q * 128:(q + 1) * 128, :, t0:t0 + TC])
                xms[("xmr", q)] = x_
            return xms

        def emit_out_tile(c, rws, i):
            """One output tile (8 accumulating matmuls) of chunk c."""
            t0 = c * TC
            ts_, eh = i // 2, i % 2
            op = ops_.tile([128, 512], dt.float32, tag="op")
            for j in range(NJ):
                nc.tensor.matmul(
                    op[:], rws[j][:, ts_ * 128:(ts_ + 1) * 128],
                    wo_t[:, j * D + eh * 512: j * D + (eh + 1) * 512],
                    start=(j == 0), stop=(j == NJ - 1))
            oc = ocp.tile([128, 512], dt.bfloat16, tag="oc")
            nc.scalar.activation(oc[:], op[:], AF.Copy)
            nc.gpsimd.dma_start(
                O[t0 + ts_ * 128: t0 + (ts_ + 1) * 128,
                  eh * 512:(eh + 1) * 512], oc[:])

        def emit_consume(prev, rws):
            """Scale/combine/divide + gate for channel group e (lagged by 1)."""
            e, ek, a, sa, sb, es = prev
            cc = cv[:, 8 + e:9 + e]
            ac = acp.tile([128, TC], dt.bfloat16, tag="ac")
            nc.scalar.activation(ac[:], a[:], AF.Copy, scale=cc)
            ekc = ekcp.tile([128, TC], dt.bfloat16, tag="ekc")
            nc.scalar.activation(ekc[:], ek[:], AF.Copy, scale=cc)
            num = nump.tile([128, TC], dt.bfloat16, tag="num")
            nc.vector.tensor_tensor(num[:], ac[:], sa[:], OP.add)
            den = denp.tile([128, TC], dt.bfloat16, tag="den")
            nc.vector.tensor_tensor(den[:], ekc[:], sb[:], OP.add)
            es2 = es2p.tile([128, TC], dt.bfloat16, tag="es2")
            nc.vector.tensor_scalar(es2[:], es[:], 1.0, None, OP.add)
            df = dfp.tile([128, TC], dt.float32, tag="df")
            nc.gpsimd.tensor_tensor(df[:], den[:], es2[:], OP.mult)
            rden = rdp.tile([128, TC], dt.float32, tag="rden")
            nc.vector.reciprocal_approx_fast(rden[:], df[:])
            rw = rwp.tile([128, TC], dt.bfloat16, tag="rw")
            nc.gpsimd.tensor_tensor(rw[:], num[:], rden[:], OP.mult)
            rws.append(rw)

        def chunk_phase(c, xms, rws_prev, c_prev, rws_prev2):
            """k/v/r matmuls + WKV recurrence + gate; interleaves out tiles
            of chunk c-1. One uniform per-e pipeline, single ACT table."""
            rws = []
            prev = None
            for e in range(NJ):
                kp = kps.tile([128, TC], dt.float32, tag="kp")
                for j in range(NJ):
                    nc.tensor.matmul(
                        kp[:], wk_t[:, j * D + e * 128: j * D + (e + 1) * 128],
                        xms[("xmk", j)][:], start=(j == 0), stop=(j == NJ - 1))
                vp = vps.tile([128, TC], dt.float32, tag="vp")
                for j in range(NJ):
                    nc.tensor.matmul(
                        vp[:], wv_t[:, j * D + e * 128: j * D + (e + 1) * 128],
                        xms[("xmv", j)][:], start=(j == 0), stop=(j == NJ - 1))
                rp = rps.tile([128, TC], dt.float32, tag="rp")
                for q in range(NJ // 2):
                    nc.tensor.matmul(
                        rp[:], wr_t[:, q * NJ + e, :, :], xms[("xmr", q)][:],
                        start=(q == 0), stop=(q == NJ // 2 - 1),
                        perf_mode=mybir.MatmulPerfMode.DoubleRow)
                if e < 2:
                    if rws_prev2 is not None:
                        emit_out_tile(c_prev - 1, rws_prev2, 6 + e)
                elif rws_prev is not None:
                    emit_out_tile(c_prev, rws_prev, e - 2)
                ek = ekp.tile([128, TC], dt.bfloat16, tag="ek")
                nc.scalar.activation(ek[:], kp[:], AF.Exp)
                vs = vsp.tile([128, TC], dt.bfloat16, tag="vs")
                nc.scalar.activation(vs[:], vp[:], AF.Copy)
                es = esp.tile([128, TC], dt.bfloat16, tag="es")
                nc.scalar.activation(es[:], rp[:], AF.Exp, scale=-1.0)
                if prev is not None:
                    emit_consume(prev, rws)
                a = ap_.tile([128, TC], dt.bfloat16, tag="a")
                nc.vector.tensor_tensor(a[:], ek[:], vs[:], OP.mult)
                ewb = cv[:, e:e + 1].broadcast_to([128, TC])
                sa = sap.tile([128, TC], dt.bfloat16, tag="sa")
                nc.vector.tensor_tensor_scan(sa[:], ewb, a[:], ast[e][:],
                                             OP.mult, OP.add)
                nc.vector.tensor_copy(ast[e][:], sa[:, TC - 1:TC])
                sb = sbp.tile([128, TC], dt.bfloat16, tag="sb")
                nc.vector.tensor_tensor_scan(sb[:], ewb, ek[:], bst[e][:],
                                             OP.mult, OP.add)
                nc.vector.tensor_copy(bst[e][:], sb[:, TC - 1:TC])
                prev = (e, ek, a, sa, sb, es)
            emit_consume(prev, rws)
            return rws

        # ---- pipelined chunk loop ----
        xms = {}
        for j in range(NJ):
            x_ = xmp.tile([128, TC], dt.bfloat16, tag=f"xmk{j}", name=f"xmk{j}")
            nc.sync.dma_start(x_[:], XK[j * 128:(j + 1) * 128, 0:TC])
            xms[("xmk", j)] = x_
        nc.sync.dma_start(wv_t[:], WV)
        for j in range(NJ):
            x_ = xmp.tile([128, TC], dt.bfloat16, tag=f"xmv{j}", name=f"xmv{j}")
            nc.sync.dma_start(x_[:], XV[j * 128:(j + 1) * 128, 0:TC])
            xms[("xmv", j)] = x_
        for q in range(NJ // 2):
            x_ = xmp.tile([128, 2, TC], dt.float8e4, tag=f"xmr{q}", name=f"xmr{q}")
            nc.sync.dma_start(x_[:], XR[q * 128:(q + 1) * 128, :, 0:TC])
            xms[("xmr", q)] = x_
        load_weights_rest()
        rws_prev = None
        rws_prev2 = None
        for c in range(nch):
            if c + 1 < nch:
                xms_n = load_xm(c + 1)
            rws = chunk_phase(c, xms, rws_prev, c - 1, rws_prev2)
            rws_prev2 = rws_prev
            rws_prev = rws
            if c + 1 < nch:
                xms = xms_n
        for i in range(6, 2 * NTS):
            emit_out_tile(nch - 2, rws_prev2, i)
        for i in range(2 * NTS):
            emit_out_tile(nch - 1, rws_prev, i)


def pack_inputs(x_slice, time_decay, time_first, time_mix_k, time_mix_v,
                time_mix_r, Wk, Wv, Wr, Wo):
    """Host-side packing for one core. x_slice: [T, D] fp32."""
    import ml_dtypes
    bf16 = ml_dtypes.bfloat16

    def packw(W):
        return np.ascontiguousarray(
            W.T.reshape(NJ, 128, D).transpose(1, 0, 2).reshape(128, NJ * D)
        ).astype(bf16)

    def packw_e(W):
        return np.ascontiguousarray(
            W.reshape(NJ, 128, NJ, 128).transpose(3, 0, 2, 1).reshape(128, NJ * D)
        ).astype(bf16)

    def packv(v):
        return np.ascontiguousarray(v.reshape(NJ, 128).T).astype(np.float32)

    x = np.asarray(x_slice, dtype=np.float32)
    T = x.shape[0]
    xprev = np.zeros_like(x)
    xprev[1:] = x[:-1]

    mk = time_mix_k.reshape(D).astype(np.float32)
    mv = time_mix_v.reshape(D).astype(np.float32)
    mr = time_mix_r.reshape(D).astype(np.float32)

    def mix(m):
        return np.ascontiguousarray((x * m + xprev * (1.0 - m)).T).astype(bf16)

    fp8 = ml_dtypes.float8_e4m3

    def mix8(m):
        xm = (x * m + xprev * (1.0 - m)).T  # [D, T] fp32
        T_ = xm.shape[1]
        return np.ascontiguousarray(
            xm.reshape(NJ // 2, 2, 128, T_).transpose(0, 2, 1, 3)
            .reshape(D // 2, 2, T_)).astype(fp8)

    def packw8(W):
        r = W.reshape(NJ, 128, NJ // 2, 2, 128)  # [e, m, q, i, p]
        return np.ascontiguousarray(
            r.transpose(4, 2, 0, 3, 1).reshape(128, NJ * NJ // 2, 2, 128)
        ).astype(fp8)

    ew = np.exp(-np.exp(time_decay.astype(np.float64)))
    cc = ew * np.exp(time_first.astype(np.float64)) - 1.0
    cv = np.concatenate(
        [packv(ew.astype(np.float32)), packv(cc.astype(np.float32))],
        axis=1).astype(np.float32)
    return {
        "xmk": mix(mk), "xmv": mix(mv), "xmr": mix8(mr),
        "wk": packw(Wk), "wv": packw(Wv), "wr": packw8(Wr), "wo": packw(Wo),
        "cv": cv,
    }


# ---------------------------------------------------------------------------
# Harness entry point: full inputs in, full output out, 8-way batch-parallel.
# ---------------------------------------------------------------------------
_CACHE = {}
_last_exec_time_ns = None


def _get_program(n_cores):
    key = ("prog", n_cores)
    if key not in _CACHE:
        nc = bacc.Bacc("TRN2", target_bir_lowering=False, debug=False,
                       num_devices=n_cores)
        build(nc, T=4096)
        nc.compile()
        _CACHE[key] = nc
    return _CACHE[key]


def kernel(x, time_decay, time_first, time_mix_k, time_mix_v, time_mix_r,
           Wk, Wv, Wr, Wo):
    """WKV attention: x [8, 4096, 1024] fp32 -> out [8, 4096, 1024] fp32.

    Shards batch across the 8 NeuronCores (one batch element per core).
    """
    global _last_exec_time_ns
    import os
    from concourse import bass_utils

    x = np.asarray(x, dtype=np.float32)
    B = x.shape[0]
    td = np.asarray(time_decay)
    tf = np.asarray(time_first)
    args = (td, tf, np.asarray(time_mix_k), np.asarray(time_mix_v),
            np.asarray(time_mix_r), np.asarray(Wk), np.asarray(Wv),
            np.asarray(Wr), np.asarray(Wo))
    in_maps = [pack_inputs(x[b], *args) for b in range(B)]

    nc = _get_program(B)
    trace = os.environ.get("WKV_TRACE", "0") == "1"
    r = bass_utils.run_bass_kernel_spmd(nc, in_maps, core_ids=list(range(B)),
                                        trace=trace)
    _last_exec_time_ns = r.exec_time_ns
    return np.stack([r.results[b]["o"] for b in range(B)]).astype(np.float32)
